# revision 38
# baseline (speedup 1.0000x reference)
"""Multi-head attention (H=16, d_model=1024, N=2048) on 8 TRN2 NeuronCores.

Sharding: tensor-parallel over heads — each core owns 2 heads (a 128-wide
slice of the QKV projection output dim and of Wo's input dim). Activations
are replicated (shipped pre-transposed by the host); each core returns its
2 heads' attention weights and a partial output projection which the host
reduces (the tensor-parallel all-reduce).

Device pipeline per core:
  1. qT,kT = (Wslice^T @ inT) -> [128(dims), 2048(tokens)] fp32 PSUM accum
     (query/key and their weight slices ship as fp16 — >= fp32r's own
     mantissa — halving the replicated-input DMA);  v = [2048, 128] from a
     bf16 value path (linear contribution only, attn numerics untouched).
  2. pass A: scores[n,m] tiles (fp32r matmuls, 1/sqrt(dk) folded into Wq on
     host) -> exp on ScalarE with fused accum_out row-sums -> per-partition
     reciprocal scale on VectorE -> contiguous attn row DMA to HBM.
  3. pass B: scores^T[m,n] tiles -> exp -> matmul with v accumulating
     x^T[dims, n] unnormalized, 512-wide n-chunks, interleaved with pass A
     so ScalarE (the bottleneck engine: 2x 8.4M-elem exp) never idles.
  4. out_partial = x^T.T @ Wo_slice per head; the pass-A row-sum
     reciprocals are applied as per-partition scalars during PSUM eviction
     and the two heads are summed (softmax normalization commutes with the
     value/output matmuls, so it is folded into these cheap evictions
     instead of touching the 2048x2048 score matrices a second time).

Engine budget per core (cost model): DMA 155us (the 33.5MB attention
output write dominates), ScalarE 151us (gapless window), TensorE 123us,
VectorE 100us; total ~220us = input prefix 50us + ScalarE window 164us +
7us drain tail.
"""

import os
import sys

import numpy as np

for _p in ("/opt/trn_rl_repo", "/opt/pypackages"):
    if os.path.isdir(_p) and _p not in sys.path:
        sys.path.append(_p)

import concourse.bass as bass
import concourse.mybir as mybir
import concourse.tile as tile
from concourse import bacc
from concourse.bass_utils import run_bass_kernel_spmd

F32 = mybir.dt.float32
F32R = mybir.dt.float32r
BF16 = mybir.dt.bfloat16
F16 = mybir.dt.float16
EXP = mybir.ActivationFunctionType.Exp

N = 2048          # tokens
D = 1024          # d_model
H = 16            # heads
DK = 64           # head dim
NCORES = 8
HC = H // NCORES  # heads per core = 2
DC = HC * DK      # projection slice per core = 128
P = 128
KO = D // P       # 8 contraction tiles over d_model
NT = N // P       # 16 token tiles
TEMP = float(np.sqrt(DK))


def _r(ap):
    return ap.bitcast(F32R)


def build_nc():
    nc = bacc.Bacc(
        "TRN2", target_bir_lowering=False, debug=False, num_devices=NCORES
    )
    qTin = nc.dram_tensor("qTin", [D, N], F16, kind="ExternalInput")
    kTin = nc.dram_tensor("kTin", [D, N], F16, kind="ExternalInput")
    vTin = nc.dram_tensor("vTin", [D, N], BF16, kind="ExternalInput")
    wq = nc.dram_tensor("wq", [P, KO, DC], F16, kind="ExternalInput")
    wk = nc.dram_tensor("wk", [P, KO, DC], F16, kind="ExternalInput")
    wv = nc.dram_tensor("wv", [P, KO, DC], BF16, kind="ExternalInput")
    wo = nc.dram_tensor("wo", [DC, D], F32, kind="ExternalInput")
    bq = nc.dram_tensor("bq", [DC, 1], F32, kind="ExternalInput")
    bk = nc.dram_tensor("bk", [DC, 1], F32, kind="ExternalInput")
    bv = nc.dram_tensor("bv", [1, DC], F32, kind="ExternalInput")
    attn = nc.dram_tensor("attn", [HC, N, N], F32, kind="ExternalOutput")
    pout = nc.dram_tensor("pout", [N, D], F32, kind="ExternalOutput")

    with tile.TileContext(nc) as tc:
        _body(nc, tc, qTin, kTin, vTin, wq, wk, wv, wo, bq, bk, bv, attn, pout)
    nc.compile()
    return nc


def _body(nc, tc, qTin, kTin, vTin, wq, wk, wv, wo, bq, bk, bv, attn, pout):
    with (
        tc.tile_pool(name="wpool", bufs=1) as wpool,
        tc.tile_pool(name="big", bufs=1) as big,
        tc.tile_pool(name="stream", bufs=3) as stream,
        tc.tile_pool(name="expp", bufs=5) as expp,
        tc.tile_pool(name="etp", bufs=6) as etp,
        tc.tile_pool(name="outp", bufs=3) as outp,
        tc.tile_pool(name="small", bufs=1) as small,
    ):
        # ---- weights / constants -------------------------------------
        wq_sb = wpool.tile([P, KO, DC], F16)
        nc.sync.dma_start(wq_sb[:], wq.ap())
        wk_sb = wpool.tile([P, KO, DC], F16)
        nc.sync.dma_start(wk_sb[:], wk.ap())
        wv_sb = wpool.tile([P, KO, DC], BF16)
        nc.sync.dma_start(wv_sb[:], wv.ap())
        # per-head Wo slices at partition base 0 (DMA places partitions freely)
        wo_h = []
        for h in range(HC):
            woh = wpool.tile([DK, D], F32, name=f"wo{h}", tag=f"wo{h}")
            nc.sync.dma_start(_r(woh[:]), _r(wo.ap()[h * DK : (h + 1) * DK, :]))
            wo_h.append(woh)
        bq_sb = wpool.tile([P, 1], F32)
        nc.sync.dma_start(bq_sb[:], bq.ap())
        bk_sb = wpool.tile([P, 1], F32)
        nc.sync.dma_start(bk_sb[:], bk.ap())
        bv_sb = wpool.tile([1, DC], F32)
        nc.sync.dma_start(bv_sb[:], bv.ap())
        ones_sb = wpool.tile([1, P], F32)
        nc.vector.memset(ones_sb[:], 1.0)

        reps = int(os.environ.get("KERNEL_BENCH_REPS", "1"))
        qT_sb = big.tile([P, N], F32)   # [dims, tokens] both heads stacked
        kT_sb = big.tile([P, N], F32)
        v_sb = big.tile([P, NT, DC], F32)  # [m_in_tile, m_tile, dims]
        # per-head unnormalized attn@v, transposed, at partition base 0
        xT_h = [
            big.tile([DK, N], F32, name=f"xT{h}", tag=f"xT{h}") for h in range(HC)
        ]
        rs_sb = big.tile([P, 2 * NT], F32)  # exp row sums, col = h*NT + nt
        rr_sb = big.tile([P, 2 * NT], F32)  # reciprocals of rs_sb

        # ---- phase 1: projections (v first so its PSUM frees early) ---
        for _rep in range(reps):
         with (
            tc.tile_pool(name="ps1", bufs=1, space="PSUM") as ps1,
            tc.tile_pool(name="psv", bufs=4, space="PSUM") as psv,
        ):
            # v: [m, dc] accumulated over 8 d_model tiles; bias via K=1 matmul
            pv = [psv.tile([P, 4, DC], F32, tag="pv", name=f"pv{b}") for b in range(4)]
            for ko2 in range(KO // 2):
                st = stream.tile([P, 2, N], BF16, tag="stv")
                nc.sync.dma_start(
                    st[:],
                    vTin.ap()[ko2 * 2 * P : (ko2 + 1) * 2 * P, :].rearrange(
                        "(t p) n -> p t n", p=P
                    ),
                )
                for t in range(2):
                    ko = ko2 * 2 + t
                    for mt in range(NT):
                        # one accumulation group per bank: start only on the
                        # bank's first matmul (start zeroes the whole bank)
                        nc.tensor.matmul(
                            pv[mt // 4][:, mt % 4, :],
                            st[:, t, mt * P : (mt + 1) * P],
                            wv_sb[:, ko, :],
                            start=(ko == 0 and mt % 4 == 0),
                            stop=False,
                        )
            for mt in range(NT):
                nc.tensor.matmul(
                    pv[mt // 4][:, mt % 4, :],
                    ones_sb[:, :P],
                    bv_sb[:],
                    start=False,
                    stop=(mt % 4 == 3),
                )
            for b in range(4):
                nc.vector.tensor_copy(_r(v_sb[:, b * 4 : (b + 1) * 4, :]), pv[b][:])

            # qT / kT: [dc, n] accumulated over 8 d_model tiles
            for name, win, wsb, bsb, dst in (
                ("q", qTin, wq_sb, bq_sb, qT_sb),
                ("k", kTin, wk_sb, bk_sb, kT_sb),
            ):
                ps_qk = ps1.tile([P, N], F32, tag="ps_qk")
                for ko2 in range(KO // 2):
                    st = stream.tile([P, 2, N], F16, tag="st")
                    nc.sync.dma_start(
                        st[:],
                        win.ap()[ko2 * 2 * P : (ko2 + 1) * 2 * P, :].rearrange(
                            "(t p) n -> p t n", p=P
                        ),
                    )
                    for t in range(2):
                        ko = ko2 * 2 + t
                        for ch in range(4):
                            nc.tensor.matmul(
                                ps_qk[:, ch * 512 : (ch + 1) * 512],
                                wsb[:, ko, :],
                                st[:, t, ch * 512 : (ch + 1) * 512],
                                start=(ko == 0),
                                stop=(ko == KO - 1),
                            )
                # chunked eviction so pass A can start per m-chunk sooner
                for ch in range(4):
                    nc.vector.tensor_scalar_add(
                        _r(dst[:, ch * 512 : (ch + 1) * 512]),
                        ps_qk[:, ch * 512 : (ch + 1) * 512],
                        bsb[:],
                    )

         # ---- phase 2+3: attention + output projection, interleaved ----
         # psA: pass-A scores [n,m] half-tiles;  psT/psX: pass-B transposed
         # scores and x^T accumulators (512-wide n chunks);  psO: out-proj.
         with (
            tc.tile_pool(name="psA", bufs=1, space="PSUM") as psA,
            tc.tile_pool(name="psT", bufs=1, space="PSUM") as psT,
            tc.tile_pool(name="psX", bufs=1, space="PSUM") as psX,
            tc.tile_pool(name="psO", bufs=2, space="PSUM") as psO,
        ):

            def pass_a(i):
                # normal-orientation attn rows nt=i for both heads
                for h in range(HC):
                    b0 = h * DK
                    qh = qT_sb[b0 : b0 + DK, :]
                    kh = kT_sb[b0 : b0 + DK, :]
                    idx = h * NT + i
                    ex = expp.tile([P, N], F32, tag="ex", name="ex")
                    rs2 = small.tile([P, 2], F32, tag="rs2", bufs=4, name="rs2")
                    for half in range(2):
                        pa = psA.tile([P, 1024], F32, tag="pa", name="pa")
                        for sub in range(2):
                            mc = half * 2 + sub
                            nc.tensor.matmul(
                                pa[:, sub * 512 : (sub + 1) * 512],
                                _r(qh[:, i * P : (i + 1) * P]),
                                _r(kh[:, mc * 512 : (mc + 1) * 512]),
                                start=True,
                                stop=True,
                            )
                        nc.scalar.activation(
                            ex[:, half * 1024 : (half + 1) * 1024],
                            pa[:],
                            EXP,
                            accum_out=rs2[:, half : half + 1],
                        )
                    nc.vector.tensor_add(
                        rs_sb[:, idx : idx + 1], rs2[:, 0:1], rs2[:, 1:2]
                    )
                    nc.vector.reciprocal(
                        rr_sb[:, idx : idx + 1], rs_sb[:, idx : idx + 1]
                    )
                    nc.vector.tensor_scalar_mul(
                        ex[:], ex[:], rr_sb[:, idx : idx + 1]
                    )
                    nc.sync.dma_start(attn.ap()[h, i * P : (i + 1) * P, :], ex[:])

            def pass_b_seg(j, px, i0, i1):
                # transposed scores for n-chunk j (512 wide), m-tiles
                # [i0,i1).  Both heads share one PSUM tile so the exp is a
                # single 1024-wide ScalarE op per m-tile.
                nb = j * 512
                for i in range(i0, i1):
                    pt = psT.tile([P, HC, 512], F32, tag="pt", name="pt")
                    for h in range(HC):
                        b0 = h * DK
                        nc.tensor.matmul(
                            pt[:, h, :],
                            _r(kT_sb[b0 : b0 + DK, i * P : (i + 1) * P]),
                            _r(qT_sb[b0 : b0 + DK, nb : nb + 512]),
                            start=True,
                            stop=True,
                        )
                    et = etp.tile([P, HC, 512], F32, tag="et", name="et")
                    nc.scalar.activation(_r(et[:]), pt[:], EXP)
                    for h in range(HC):
                        b0 = h * DK
                        nc.tensor.matmul(
                            px[:, h, :],
                            _r(v_sb[:, i, b0 : b0 + DK]),
                            _r(et[:, h, :]),
                            start=(i == 0),
                            stop=(i == NT - 1),
                        )

            def out_proj(i, tail=False):
                # out rows nt=i: matmul per head, normalize by pass-A row
                # sums during eviction, combine, store. In the kernel tail
                # (last block) spread the evictions over the idle ACT and
                # GpSimd engines instead of serializing on DVE.
                for ch in range(2):
                    po = [None, None]
                    for h in range(HC):
                        po[h] = psO.tile([P, 512], F32, tag="po", name=f"po{h}")
                        nc.tensor.matmul(
                            po[h][:],
                            _r(xT_h[h][:, i * P : (i + 1) * P]),
                            _r(wo_h[h][:, ch * 512 : (ch + 1) * 512]),
                            start=True,
                            stop=True,
                        )
                    ob = outp.tile([P, 512], F32, tag="ob", name="ob")
                    nc.vector.tensor_scalar_mul(ob[:], po[0][:], rr_sb[:, i : i + 1])
                    ob2 = outp.tile([P, 512], F32, tag="ob2", name="ob2")
                    if tail:
                        nc.scalar.mul(ob2[:], po[1][:], rr_sb[:, NT + i : NT + i + 1])
                        nc.gpsimd.tensor_add(ob[:], ob[:], ob2[:])
                    else:
                        nc.vector.tensor_scalar_mul(
                            ob2[:], po[1][:], rr_sb[:, NT + i : NT + i + 1]
                        )
                        nc.vector.tensor_add(ob[:], ob[:], ob2[:])
                    nc.sync.dma_start(
                        pout.ap()[i * P : (i + 1) * P, ch * 512 : (ch + 1) * 512],
                        ob[:],
                    )

            # interleave: alternate pass-A tiles with pass-B segments so
            # ACT always has ready work; out-proj as soon as its n-chunk of
            # x^T completes
            for blk in range(4):
                px = psX.tile([DK, HC, 512], F32, tag="px", name="px")
                for ii in range(4):
                    pass_b_seg(blk, px, 4 * ii, 4 * ii + 4)
                    pass_a(4 * blk + ii)
                nb = blk * 512
                for h in range(HC):
                    nc.vector.tensor_copy(
                        _r(xT_h[h][:, nb : nb + 512]), px[:, h, :]
                    )
                for i in range(4 * blk, 4 * blk + 4):
                    out_proj(i, tail=(blk == 3))


def make_in_maps(query, key, value, Wq, bq, Wk, bk, Wv, bv, Wo):
    """Shard the full inputs into one input map per core."""
    f = lambda a: np.ascontiguousarray(np.asarray(a, dtype=np.float32))
    import ml_dtypes
    qT = np.ascontiguousarray(np.asarray(query, dtype=np.float32).T.astype(np.float16))
    kT = np.ascontiguousarray(np.asarray(key, dtype=np.float32).T.astype(np.float16))
    vT = np.ascontiguousarray(
        np.asarray(value, dtype=np.float32).T.astype(ml_dtypes.bfloat16)
    )
    Wq, bq, Wk, bk, Wv, bv, Wo = map(f, (Wq, bq, Wk, bk, Wv, bv, Wo))
    in_maps = []
    for c in range(NCORES):
        sl = slice(c * DC, (c + 1) * DC)
        shuf = lambda w: np.ascontiguousarray(
            w.reshape(KO, P, DC).transpose(1, 0, 2)
        )
        in_maps.append(
            {
                "qTin": qT,
                "kTin": kT,
                "vTin": vT,
                "wq": shuf((Wq[:, sl] / TEMP).astype(np.float16)),
                "wk": shuf(Wk[:, sl].astype(np.float16)),
                "wv": shuf(Wv[:, sl].astype(ml_dtypes.bfloat16)),
                "wo": f(Wo[sl, :]),
                "bq": f(bq[sl] / TEMP).reshape(DC, 1),
                "bk": f(bk[sl]).reshape(DC, 1),
                "bv": f(bv[sl]).reshape(1, DC),
            }
        )
    return in_maps


_NC_CACHE = []
LAST_RESULT = None


def kernel(query, key, value, Wq, bq, Wk, bk, Wv, bv, Wo, bo):
    global LAST_RESULT
    if not _NC_CACHE:
        _NC_CACHE.append(build_nc())
    nc = _NC_CACHE[0]
    in_maps = make_in_maps(query, key, value, Wq, bq, Wk, bk, Wv, bv, Wo)
    res = run_bass_kernel_spmd(nc, in_maps, core_ids=list(range(NCORES)))
    LAST_RESULT = res
    attn = np.concatenate([np.asarray(r["attn"]) for r in res.results], axis=0)
    out = np.sum([np.asarray(r["pout"]) for r in res.results], axis=0)
    out = (out + np.asarray(bo, dtype=np.float32)).astype(np.float32)
    return out, attn



# revision 39
# speedup vs baseline: 1.0201x; 1.0201x over previous
"""Multi-head attention (H=16, d_model=1024, N=2048) on 8 TRN2 NeuronCores.

Sharding: tensor-parallel over heads — each core owns 2 heads (a 128-wide
slice of the QKV projection output dim and of Wo's input dim). Activations
are replicated (shipped pre-transposed by the host); each core returns its
2 heads' attention weights and a partial output projection which the host
reduces (the tensor-parallel all-reduce).

Device pipeline per core:
  1. qT,kT = (Wslice^T @ inT) -> [128(dims), 2048(tokens)] fp32 PSUM accum
     (query/key and their weight slices ship as fp16 — >= fp32r's own
     mantissa — halving the replicated-input DMA);  v = [2048, 128] from a
     bf16 value path (linear contribution only, attn numerics untouched).
  2. pass A: scores[n,m] tiles (fp32r matmuls, 1/sqrt(dk) folded into Wq on
     host) -> exp on ScalarE with fused accum_out row-sums -> per-partition
     reciprocal scale on VectorE -> contiguous attn row DMA to HBM.
  3. pass B: scores^T[m,n] tiles -> exp -> matmul with v accumulating
     x^T[dims, n] unnormalized, 512-wide n-chunks, interleaved with pass A
     so ScalarE (the bottleneck engine: 2x 8.4M-elem exp) never idles.
  4. out_partial = x^T.T @ Wo_slice per head; the pass-A row-sum
     reciprocals are applied as per-partition scalars during PSUM eviction
     and the two heads are summed (softmax normalization commutes with the
     value/output matmuls, so it is folded into these cheap evictions
     instead of touching the 2048x2048 score matrices a second time).

Engine budget per core (cost model): DMA 155us (the 33.5MB attention
output write dominates), ScalarE 151us (gapless window), TensorE 123us,
VectorE 100us; total ~220us = input prefix 50us + ScalarE window 164us +
7us drain tail.
"""

import os
import sys

import numpy as np

for _p in ("/opt/trn_rl_repo", "/opt/pypackages"):
    if os.path.isdir(_p) and _p not in sys.path:
        sys.path.append(_p)

import concourse.bass as bass
import concourse.mybir as mybir
import concourse.tile as tile
from concourse import bacc
from concourse.bass_utils import run_bass_kernel_spmd

F32 = mybir.dt.float32
F32R = mybir.dt.float32r
BF16 = mybir.dt.bfloat16
F16 = mybir.dt.float16
EXP = mybir.ActivationFunctionType.Exp

N = 2048          # tokens
D = 1024          # d_model
H = 16            # heads
DK = 64           # head dim
NCORES = 8
HC = H // NCORES  # heads per core = 2
DC = HC * DK      # projection slice per core = 128
P = 128
KO = D // P       # 8 contraction tiles over d_model
NT = N // P       # 16 token tiles
TEMP = float(np.sqrt(DK))


def _r(ap):
    return ap.bitcast(F32R)


def build_nc():
    nc = bacc.Bacc(
        "TRN2", target_bir_lowering=False, debug=False, num_devices=NCORES
    )
    qTin = nc.dram_tensor("qTin", [D, N], F16, kind="ExternalInput")
    kTin = nc.dram_tensor("kTin", [D, N], F16, kind="ExternalInput")
    vTin = nc.dram_tensor("vTin", [D, N], BF16, kind="ExternalInput")
    wq = nc.dram_tensor("wq", [P, KO, DC], F16, kind="ExternalInput")
    wk = nc.dram_tensor("wk", [P, KO, DC], F16, kind="ExternalInput")
    wv = nc.dram_tensor("wv", [P, KO, DC], BF16, kind="ExternalInput")
    wo = nc.dram_tensor("wo", [DC, D], F32, kind="ExternalInput")
    bq = nc.dram_tensor("bq", [DC, 1], F32, kind="ExternalInput")
    bk = nc.dram_tensor("bk", [DC, 1], F32, kind="ExternalInput")
    bv = nc.dram_tensor("bv", [1, DC], F32, kind="ExternalInput")
    attn = nc.dram_tensor("attn", [HC, N, N], F32, kind="ExternalOutput")
    pout = nc.dram_tensor("pout", [N, D], F32, kind="ExternalOutput")

    with tile.TileContext(nc) as tc:
        _body(nc, tc, qTin, kTin, vTin, wq, wk, wv, wo, bq, bk, bv, attn, pout)
    nc.compile()
    return nc


def _body(nc, tc, qTin, kTin, vTin, wq, wk, wv, wo, bq, bk, bv, attn, pout):
    with (
        tc.tile_pool(name="wpool", bufs=1) as wpool,
        tc.tile_pool(name="big", bufs=1) as big,
        tc.tile_pool(name="stream", bufs=3) as stream,
        tc.tile_pool(name="expp", bufs=5) as expp,
        tc.tile_pool(name="etp", bufs=6) as etp,
        tc.tile_pool(name="outp", bufs=3) as outp,
        tc.tile_pool(name="small", bufs=1) as small,
    ):
        # ---- weights / constants -------------------------------------
        wq_sb = wpool.tile([P, KO, DC], F16)
        nc.sync.dma_start(wq_sb[:], wq.ap())
        wk_sb = wpool.tile([P, KO, DC], F16)
        nc.sync.dma_start(wk_sb[:], wk.ap())
        wv_sb = wpool.tile([P, KO, DC], BF16)
        nc.sync.dma_start(wv_sb[:], wv.ap())
        # per-head Wo slices at partition base 0 (DMA places partitions freely)
        wo_h = []
        for h in range(HC):
            woh = wpool.tile([DK, D], F32, name=f"wo{h}", tag=f"wo{h}")
            nc.sync.dma_start(_r(woh[:]), _r(wo.ap()[h * DK : (h + 1) * DK, :]))
            wo_h.append(woh)
        bq_sb = wpool.tile([P, 1], F32)
        nc.sync.dma_start(bq_sb[:], bq.ap())
        bk_sb = wpool.tile([P, 1], F32)
        nc.sync.dma_start(bk_sb[:], bk.ap())
        bv_sb = wpool.tile([1, DC], F32)
        nc.sync.dma_start(bv_sb[:], bv.ap())
        ones_sb = wpool.tile([1, P], F32)
        nc.vector.memset(ones_sb[:], 1.0)

        reps = int(os.environ.get("KERNEL_BENCH_REPS", "1"))
        qT_sb = big.tile([P, N], F32)   # [dims, tokens] both heads stacked
        kT_sb = big.tile([P, N], F32)
        v_sb = big.tile([P, NT, DC], F32)  # [m_in_tile, m_tile, dims]
        # per-head unnormalized attn@v, transposed, at partition base 0
        xT_h = [
            big.tile([DK, N], F32, name=f"xT{h}", tag=f"xT{h}") for h in range(HC)
        ]
        rs_sb = big.tile([P, 2 * NT], F32)  # exp row sums, col = h*NT + nt
        rr_sb = big.tile([P, 2 * NT], F32)  # reciprocals of rs_sb

        # ---- phase 1: projections (v first so its PSUM frees early) ---
        for _rep in range(reps):
         with tc.tile_pool(name="ps1", bufs=1, space="PSUM") as ps1:
            # qT / kT: [dc, n] accumulated over 8 d_model tiles
            for name, win, wsb, bsb, dst in (
                ("q", qTin, wq_sb, bq_sb, qT_sb),
                ("k", kTin, wk_sb, bk_sb, kT_sb),
            ):
                ps_qk = ps1.tile([P, N], F32, tag="ps_qk")
                for ko2 in range(KO // 2):
                    st = stream.tile([P, 2, N], F16, tag="st")
                    nc.sync.dma_start(
                        st[:],
                        win.ap()[ko2 * 2 * P : (ko2 + 1) * 2 * P, :].rearrange(
                            "(t p) n -> p t n", p=P
                        ),
                    )
                    for t in range(2):
                        ko = ko2 * 2 + t
                        for ch in range(4):
                            nc.tensor.matmul(
                                ps_qk[:, ch * 512 : (ch + 1) * 512],
                                wsb[:, ko, :],
                                st[:, t, ch * 512 : (ch + 1) * 512],
                                start=(ko == 0),
                                stop=(ko == KO - 1),
                            )
                # chunked eviction so pass A can start per m-chunk sooner
                for ch in range(4):
                    nc.vector.tensor_scalar_add(
                        _r(dst[:, ch * 512 : (ch + 1) * 512]),
                        ps_qk[:, ch * 512 : (ch + 1) * 512],
                        bsb[:],
                    )

         # ---- phase 2+3: attention + output projection, interleaved ----
         # psA: pass-A scores [n,m] half-tiles;  psT/psX: pass-B transposed
         # scores and x^T accumulators (512-wide n chunks);  psO: out-proj.
         with (
            tc.tile_pool(name="psA", bufs=1, space="PSUM") as psA,
            tc.tile_pool(name="psT", bufs=1, space="PSUM") as psT,
            tc.tile_pool(name="psX", bufs=1, space="PSUM") as psX,
            tc.tile_pool(name="psO", bufs=2, space="PSUM") as psO,
        ):

            def pass_a(i):
                # normal-orientation attn rows nt=i for both heads
                for h in range(HC):
                    b0 = h * DK
                    qh = qT_sb[b0 : b0 + DK, :]
                    kh = kT_sb[b0 : b0 + DK, :]
                    idx = h * NT + i
                    ex = expp.tile([P, N], F32, tag="ex", name="ex")
                    rs2 = small.tile([P, 2], F32, tag="rs2", bufs=4, name="rs2")
                    for half in range(2):
                        pa = psA.tile([P, 1024], F32, tag="pa", name="pa")
                        for sub in range(2):
                            mc = half * 2 + sub
                            nc.tensor.matmul(
                                pa[:, sub * 512 : (sub + 1) * 512],
                                _r(qh[:, i * P : (i + 1) * P]),
                                _r(kh[:, mc * 512 : (mc + 1) * 512]),
                                start=True,
                                stop=True,
                            )
                        nc.scalar.activation(
                            ex[:, half * 1024 : (half + 1) * 1024],
                            pa[:],
                            EXP,
                            accum_out=rs2[:, half : half + 1],
                        )
                    nc.vector.tensor_add(
                        rs_sb[:, idx : idx + 1], rs2[:, 0:1], rs2[:, 1:2]
                    )
                    nc.vector.reciprocal(
                        rr_sb[:, idx : idx + 1], rs_sb[:, idx : idx + 1]
                    )
                    nc.vector.tensor_scalar_mul(
                        ex[:], ex[:], rr_sb[:, idx : idx + 1]
                    )
                    nc.sync.dma_start(attn.ap()[h, i * P : (i + 1) * P, :], ex[:])

            def pass_b_seg(j, px, i0, i1):
                # transposed scores for n-chunk j (512 wide), m-tiles
                # [i0,i1).  Both heads share one PSUM tile so the exp is a
                # single 1024-wide ScalarE op per m-tile.
                nb = j * 512
                for i in range(i0, i1):
                    pt = psT.tile([P, HC, 512], F32, tag="pt", name="pt")
                    for h in range(HC):
                        b0 = h * DK
                        nc.tensor.matmul(
                            pt[:, h, :],
                            _r(kT_sb[b0 : b0 + DK, i * P : (i + 1) * P]),
                            _r(qT_sb[b0 : b0 + DK, nb : nb + 512]),
                            start=True,
                            stop=True,
                        )
                    et = etp.tile([P, HC, 512], F32, tag="et", name="et")
                    nc.scalar.activation(_r(et[:]), pt[:], EXP)
                    for h in range(HC):
                        b0 = h * DK
                        nc.tensor.matmul(
                            px[:, h, :],
                            _r(v_sb[:, i, b0 : b0 + DK]),
                            _r(et[:, h, :]),
                            start=(i == 0),
                            stop=(i == NT - 1),
                        )

            def out_proj(i, tail=False):
                # out rows nt=i: matmul per head, normalize by pass-A row
                # sums during eviction, combine, store. In the kernel tail
                # (last block) spread the evictions over the idle ACT and
                # GpSimd engines instead of serializing on DVE.
                for ch in range(2):
                    po = [None, None]
                    for h in range(HC):
                        po[h] = psO.tile([P, 512], F32, tag="po", name=f"po{h}")
                        nc.tensor.matmul(
                            po[h][:],
                            _r(xT_h[h][:, i * P : (i + 1) * P]),
                            _r(wo_h[h][:, ch * 512 : (ch + 1) * 512]),
                            start=True,
                            stop=True,
                        )
                    ob = outp.tile([P, 512], F32, tag="ob", name="ob")
                    nc.vector.tensor_scalar_mul(ob[:], po[0][:], rr_sb[:, i : i + 1])
                    ob2 = outp.tile([P, 512], F32, tag="ob2", name="ob2")
                    if tail:
                        nc.scalar.mul(ob2[:], po[1][:], rr_sb[:, NT + i : NT + i + 1])
                        nc.gpsimd.tensor_add(ob[:], ob[:], ob2[:])
                    else:
                        nc.vector.tensor_scalar_mul(
                            ob2[:], po[1][:], rr_sb[:, NT + i : NT + i + 1]
                        )
                        nc.vector.tensor_add(ob[:], ob[:], ob2[:])
                    nc.sync.dma_start(
                        pout.ap()[i * P : (i + 1) * P, ch * 512 : (ch + 1) * 512],
                        ob[:],
                    )

            # interleave: alternate pass-A tiles with pass-B segments so
            # ACT always has ready work; out-proj as soon as its n-chunk of
            # x^T completes
            # v projection inside the attention scope, borrowing the
            # px slot (2 banks, 8 m-tiles) and two po slots (1 bank, 4
            # m-tiles each) before their first use -- score/exp work on
            # q/k starts ~15us earlier and pass A (which never touches v)
            # keeps ScalarE busy while v streams in.
            pv_a = psX.tile([P, 8, DC], F32, tag="px", name="pv_a")
            pv_b = psO.tile([P, 4, DC], F32, tag="po", name="pv_b")
            pv_c = psO.tile([P, 4, DC], F32, tag="po", name="pv_c")

            def pv_slot(mt):
                if mt < 8:
                    return pv_a, mt
                if mt < 12:
                    return pv_b, mt - 8
                return pv_c, mt - 12

            for ko2 in range(KO // 2):
                st = stream.tile([P, 2, N], BF16, tag="stv")
                nc.sync.dma_start(
                    st[:],
                    vTin.ap()[ko2 * 2 * P : (ko2 + 1) * 2 * P, :].rearrange(
                        "(t p) n -> p t n", p=P
                    ),
                )
                for t in range(2):
                    ko = ko2 * 2 + t
                    for mt in range(NT):
                        pvt, j = pv_slot(mt)
                        # one accumulation group per bank: start only on the
                        # bank's first matmul (start zeroes the whole bank)
                        nc.tensor.matmul(
                            pvt[:, j, :],
                            st[:, t, mt * P : (mt + 1) * P],
                            wv_sb[:, ko, :],
                            start=(ko == 0 and j % 4 == 0),
                            stop=False,
                        )
            for mt in range(NT):
                pvt, j = pv_slot(mt)
                nc.tensor.matmul(
                    pvt[:, j, :],
                    ones_sb[:, :P],
                    bv_sb[:],
                    start=False,
                    stop=(j % 4 == 3),
                )
            nc.vector.tensor_copy(_r(v_sb[:, 0:8, :]), pv_a[:])
            nc.vector.tensor_copy(_r(v_sb[:, 8:12, :]), pv_b[:])
            nc.vector.tensor_copy(_r(v_sb[:, 12:16, :]), pv_c[:])

            for blk in range(4):
                px = psX.tile([DK, HC, 512], F32, tag="px", name="px")
                for ii in range(4):
                    pass_b_seg(blk, px, 4 * ii, 4 * ii + 4)
                    pass_a(4 * blk + ii)
                nb = blk * 512
                for h in range(HC):
                    nc.vector.tensor_copy(
                        _r(xT_h[h][:, nb : nb + 512]), px[:, h, :]
                    )
                for i in range(4 * blk, 4 * blk + 4):
                    out_proj(i, tail=(blk == 3))


def make_in_maps(query, key, value, Wq, bq, Wk, bk, Wv, bv, Wo):
    """Shard the full inputs into one input map per core."""
    f = lambda a: np.ascontiguousarray(np.asarray(a, dtype=np.float32))
    import ml_dtypes
    qT = np.ascontiguousarray(np.asarray(query, dtype=np.float32).T.astype(np.float16))
    kT = np.ascontiguousarray(np.asarray(key, dtype=np.float32).T.astype(np.float16))
    vT = np.ascontiguousarray(
        np.asarray(value, dtype=np.float32).T.astype(ml_dtypes.bfloat16)
    )
    Wq, bq, Wk, bk, Wv, bv, Wo = map(f, (Wq, bq, Wk, bk, Wv, bv, Wo))
    in_maps = []
    for c in range(NCORES):
        sl = slice(c * DC, (c + 1) * DC)
        shuf = lambda w: np.ascontiguousarray(
            w.reshape(KO, P, DC).transpose(1, 0, 2)
        )
        in_maps.append(
            {
                "qTin": qT,
                "kTin": kT,
                "vTin": vT,
                "wq": shuf((Wq[:, sl] / TEMP).astype(np.float16)),
                "wk": shuf(Wk[:, sl].astype(np.float16)),
                "wv": shuf(Wv[:, sl].astype(ml_dtypes.bfloat16)),
                "wo": f(Wo[sl, :]),
                "bq": f(bq[sl] / TEMP).reshape(DC, 1),
                "bk": f(bk[sl]).reshape(DC, 1),
                "bv": f(bv[sl]).reshape(1, DC),
            }
        )
    return in_maps


_NC_CACHE = []
LAST_RESULT = None


def kernel(query, key, value, Wq, bq, Wk, bk, Wv, bv, Wo, bo):
    global LAST_RESULT
    if not _NC_CACHE:
        _NC_CACHE.append(build_nc())
    nc = _NC_CACHE[0]
    in_maps = make_in_maps(query, key, value, Wq, bq, Wk, bk, Wv, bv, Wo)
    res = run_bass_kernel_spmd(nc, in_maps, core_ids=list(range(NCORES)))
    LAST_RESULT = res
    attn = np.concatenate([np.asarray(r["attn"]) for r in res.results], axis=0)
    out = np.sum([np.asarray(r["pout"]) for r in res.results], axis=0)
    out = (out + np.asarray(bo, dtype=np.float32)).astype(np.float32)
    return out, attn



# revision 40
# speedup vs baseline: 1.0262x; 1.0059x over previous
"""Multi-head attention (H=16, d_model=1024, N=2048) on 8 TRN2 NeuronCores.

Sharding: tensor-parallel over heads — each core owns 2 heads (a 128-wide
slice of the QKV projection output dim and of Wo's input dim). Activations
are replicated (shipped pre-transposed by the host); each core returns its
2 heads' attention weights and a partial output projection which the host
reduces (the tensor-parallel all-reduce).

Device pipeline per core:
  1. qT,kT = (Wslice^T @ inT) -> [128(dims), 2048(tokens)] fp32 PSUM accum
     (query/key and their weight slices ship as fp16 — >= fp32r's own
     mantissa — halving the replicated-input DMA);  v = [2048, 128] from a
     bf16 value path (linear contribution only, attn numerics untouched).
  2. pass A: scores[n,m] tiles (fp32r matmuls, 1/sqrt(dk) folded into Wq on
     host) -> exp on ScalarE with fused accum_out row-sums -> per-partition
     reciprocal scale on VectorE -> contiguous attn row DMA to HBM.
  3. pass B: scores^T[m,n] tiles -> exp -> matmul with v accumulating
     x^T[dims, n] unnormalized, 512-wide n-chunks, interleaved with pass A
     so ScalarE (the bottleneck engine: 2x 8.4M-elem exp) never idles.
  4. out_partial = x^T.T @ Wo_slice per head; the pass-A row-sum
     reciprocals are applied as per-partition scalars during PSUM eviction
     and the two heads are summed (softmax normalization commutes with the
     value/output matmuls, so it is folded into these cheap evictions
     instead of touching the 2048x2048 score matrices a second time).

Engine budget per core (cost model): DMA 155us (the 33.5MB attention
output write dominates), ScalarE 151us (gapless window), TensorE 123us,
VectorE 100us; total ~220us = input prefix 50us + ScalarE window 164us +
7us drain tail.
"""

import os
import sys

import numpy as np

for _p in ("/opt/trn_rl_repo", "/opt/pypackages"):
    if os.path.isdir(_p) and _p not in sys.path:
        sys.path.append(_p)

import concourse.bass as bass
import concourse.mybir as mybir
import concourse.tile as tile
from concourse import bacc
from concourse.bass_utils import run_bass_kernel_spmd

F32 = mybir.dt.float32
F32R = mybir.dt.float32r
BF16 = mybir.dt.bfloat16
F16 = mybir.dt.float16
EXP = mybir.ActivationFunctionType.Exp

N = 2048          # tokens
D = 1024          # d_model
H = 16            # heads
DK = 64           # head dim
NCORES = 8
HC = H // NCORES  # heads per core = 2
DC = HC * DK      # projection slice per core = 128
P = 128
KO = D // P       # 8 contraction tiles over d_model
NT = N // P       # 16 token tiles
TEMP = float(np.sqrt(DK))


def _r(ap):
    return ap.bitcast(F32R)


def build_nc():
    nc = bacc.Bacc(
        "TRN2", target_bir_lowering=False, debug=False, num_devices=NCORES
    )
    qTin = nc.dram_tensor("qTin", [D, N], F16, kind="ExternalInput")
    kTin = nc.dram_tensor("kTin", [D, N], F16, kind="ExternalInput")
    vTin = nc.dram_tensor("vTin", [D, N], BF16, kind="ExternalInput")
    wq = nc.dram_tensor("wq", [P, KO, DC], F16, kind="ExternalInput")
    wk = nc.dram_tensor("wk", [P, KO, DC], F16, kind="ExternalInput")
    wv = nc.dram_tensor("wv", [P, KO, DC], BF16, kind="ExternalInput")
    wo = nc.dram_tensor("wo", [DC, D], F32, kind="ExternalInput")
    bq = nc.dram_tensor("bq", [DC, 1], F32, kind="ExternalInput")
    bk = nc.dram_tensor("bk", [DC, 1], F32, kind="ExternalInput")
    bv = nc.dram_tensor("bv", [1, DC], F32, kind="ExternalInput")
    attn = nc.dram_tensor("attn", [HC, N, N], F32, kind="ExternalOutput")
    pout = nc.dram_tensor("pout", [N, D], F32, kind="ExternalOutput")

    with tile.TileContext(nc) as tc:
        _body(nc, tc, qTin, kTin, vTin, wq, wk, wv, wo, bq, bk, bv, attn, pout)
    nc.compile()
    return nc


def _body(nc, tc, qTin, kTin, vTin, wq, wk, wv, wo, bq, bk, bv, attn, pout):
    with (
        tc.tile_pool(name="wpool", bufs=1) as wpool,
        tc.tile_pool(name="big", bufs=1) as big,
        tc.tile_pool(name="stream", bufs=3) as stream,
        tc.tile_pool(name="expp", bufs=5) as expp,
        tc.tile_pool(name="etp", bufs=10) as etp,
        tc.tile_pool(name="outp", bufs=3) as outp,
        tc.tile_pool(name="small", bufs=1) as small,
    ):
        # ---- weights / constants -------------------------------------
        wq_sb = wpool.tile([P, KO, DC], F16)
        nc.sync.dma_start(wq_sb[:], wq.ap())
        wk_sb = wpool.tile([P, KO, DC], F16)
        nc.sync.dma_start(wk_sb[:], wk.ap())
        wv_sb = wpool.tile([P, KO, DC], BF16)
        nc.sync.dma_start(wv_sb[:], wv.ap())
        # per-head Wo slices at partition base 0 (DMA places partitions freely)
        wo_h = []
        for h in range(HC):
            woh = wpool.tile([DK, D], F32, name=f"wo{h}", tag=f"wo{h}")
            nc.sync.dma_start(_r(woh[:]), _r(wo.ap()[h * DK : (h + 1) * DK, :]))
            wo_h.append(woh)
        bq_sb = wpool.tile([P, 1], F32)
        nc.sync.dma_start(bq_sb[:], bq.ap())
        bk_sb = wpool.tile([P, 1], F32)
        nc.sync.dma_start(bk_sb[:], bk.ap())
        bv_sb = wpool.tile([1, DC], F32)
        nc.sync.dma_start(bv_sb[:], bv.ap())
        ones_sb = wpool.tile([1, P], F32)
        nc.vector.memset(ones_sb[:], 1.0)

        reps = int(os.environ.get("KERNEL_BENCH_REPS", "1"))
        qT_sb = big.tile([P, N], F32)   # [dims, tokens] both heads stacked
        kT_sb = big.tile([P, N], F32)
        v_sb = big.tile([P, NT, DC], F32)  # [m_in_tile, m_tile, dims]
        # per-head unnormalized attn@v, transposed, at partition base 0
        xT_h = [
            big.tile([DK, N], F32, name=f"xT{h}", tag=f"xT{h}") for h in range(HC)
        ]
        rs_sb = big.tile([P, 2 * NT], F32)  # exp row sums, col = h*NT + nt
        rr_sb = big.tile([P, 2 * NT], F32)  # reciprocals of rs_sb

        # ---- phase 1: projections (v first so its PSUM frees early) ---
        for _rep in range(reps):
         with tc.tile_pool(name="ps1", bufs=1, space="PSUM") as ps1:
            # qT / kT: [dc, n] accumulated over 8 d_model tiles
            for name, win, wsb, bsb, dst in (
                ("q", qTin, wq_sb, bq_sb, qT_sb),
                ("k", kTin, wk_sb, bk_sb, kT_sb),
            ):
                ps_qk = ps1.tile([P, N], F32, tag="ps_qk")
                for ko2 in range(KO // 2):
                    st = stream.tile([P, 2, N], F16, tag="st")
                    nc.sync.dma_start(
                        st[:],
                        win.ap()[ko2 * 2 * P : (ko2 + 1) * 2 * P, :].rearrange(
                            "(t p) n -> p t n", p=P
                        ),
                    )
                    for t in range(2):
                        ko = ko2 * 2 + t
                        for ch in range(4):
                            nc.tensor.matmul(
                                ps_qk[:, ch * 512 : (ch + 1) * 512],
                                wsb[:, ko, :],
                                st[:, t, ch * 512 : (ch + 1) * 512],
                                start=(ko == 0),
                                stop=(ko == KO - 1),
                            )
                # chunked eviction so pass A can start per m-chunk sooner
                for ch in range(4):
                    nc.vector.tensor_scalar_add(
                        _r(dst[:, ch * 512 : (ch + 1) * 512]),
                        ps_qk[:, ch * 512 : (ch + 1) * 512],
                        bsb[:],
                    )

         # ---- phase 2+3: attention + output projection, interleaved ----
         # psA: pass-A scores [n,m] half-tiles;  psT/psX: pass-B transposed
         # scores and x^T accumulators (512-wide n chunks);  psO: out-proj.
         with (
            tc.tile_pool(name="psA", bufs=1, space="PSUM") as psA,
            tc.tile_pool(name="psT", bufs=1, space="PSUM") as psT,
            tc.tile_pool(name="psX", bufs=1, space="PSUM") as psX,
            tc.tile_pool(name="psO", bufs=2, space="PSUM") as psO,
        ):

            def pass_a(i):
                # normal-orientation attn rows nt=i for both heads
                for h in range(HC):
                    b0 = h * DK
                    qh = qT_sb[b0 : b0 + DK, :]
                    kh = kT_sb[b0 : b0 + DK, :]
                    idx = h * NT + i
                    ex = expp.tile([P, N], F32, tag="ex", name="ex")
                    rs2 = small.tile([P, 2], F32, tag="rs2", bufs=4, name="rs2")
                    for half in range(2):
                        pa = psA.tile([P, 1024], F32, tag="pa", name="pa")
                        for sub in range(2):
                            mc = half * 2 + sub
                            nc.tensor.matmul(
                                pa[:, sub * 512 : (sub + 1) * 512],
                                _r(qh[:, i * P : (i + 1) * P]),
                                _r(kh[:, mc * 512 : (mc + 1) * 512]),
                                start=True,
                                stop=True,
                            )
                        nc.scalar.activation(
                            ex[:, half * 1024 : (half + 1) * 1024],
                            pa[:],
                            EXP,
                            accum_out=rs2[:, half : half + 1],
                        )
                    nc.vector.tensor_add(
                        rs_sb[:, idx : idx + 1], rs2[:, 0:1], rs2[:, 1:2]
                    )
                    nc.vector.reciprocal(
                        rr_sb[:, idx : idx + 1], rs_sb[:, idx : idx + 1]
                    )
                    nc.vector.tensor_scalar_mul(
                        ex[:], ex[:], rr_sb[:, idx : idx + 1]
                    )
                    nc.sync.dma_start(attn.ap()[h, i * P : (i + 1) * P, :], ex[:])

            def pass_b_seg(j, px, i0, i1):
                # transposed scores for n-chunk j (512 wide), m-tiles
                # [i0,i1).  Both heads share one PSUM tile so the exp is a
                # single 1024-wide ScalarE op per m-tile.
                nb = j * 512
                for i in range(i0, i1):
                    pt = psT.tile([P, HC, 512], F32, tag="pt", name="pt")
                    for h in range(HC):
                        b0 = h * DK
                        nc.tensor.matmul(
                            pt[:, h, :],
                            _r(kT_sb[b0 : b0 + DK, i * P : (i + 1) * P]),
                            _r(qT_sb[b0 : b0 + DK, nb : nb + 512]),
                            start=True,
                            stop=True,
                        )
                    et = etp.tile([P, HC, 512], F32, tag="et", name="et")
                    nc.scalar.activation(_r(et[:]), pt[:], EXP)
                    for h in range(HC):
                        b0 = h * DK
                        nc.tensor.matmul(
                            px[:, h, :],
                            _r(v_sb[:, i, b0 : b0 + DK]),
                            _r(et[:, h, :]),
                            start=(i == 0),
                            stop=(i == NT - 1),
                        )

            def out_proj(i, tail=False):
                # out rows nt=i: matmul per head, normalize by pass-A row
                # sums during eviction, combine, store. In the kernel tail
                # (last block) spread the evictions over the idle ACT and
                # GpSimd engines instead of serializing on DVE.
                for ch in range(2):
                    po = [None, None]
                    for h in range(HC):
                        po[h] = psO.tile([P, 512], F32, tag="po", name=f"po{h}")
                        nc.tensor.matmul(
                            po[h][:],
                            _r(xT_h[h][:, i * P : (i + 1) * P]),
                            _r(wo_h[h][:, ch * 512 : (ch + 1) * 512]),
                            start=True,
                            stop=True,
                        )
                    ob = outp.tile([P, 512], F32, tag="ob", name="ob")
                    nc.vector.tensor_scalar_mul(ob[:], po[0][:], rr_sb[:, i : i + 1])
                    ob2 = outp.tile([P, 512], F32, tag="ob2", name="ob2")
                    if tail:
                        nc.scalar.mul(ob2[:], po[1][:], rr_sb[:, NT + i : NT + i + 1])
                        nc.gpsimd.tensor_add(ob[:], ob[:], ob2[:])
                    else:
                        nc.vector.tensor_scalar_mul(
                            ob2[:], po[1][:], rr_sb[:, NT + i : NT + i + 1]
                        )
                        nc.vector.tensor_add(ob[:], ob[:], ob2[:])
                    nc.sync.dma_start(
                        pout.ap()[i * P : (i + 1) * P, ch * 512 : (ch + 1) * 512],
                        ob[:],
                    )

            # interleave: alternate pass-A tiles with pass-B segments so
            # ACT always has ready work; out-proj as soon as its n-chunk of
            # x^T completes
            # v projection inside the attention scope, borrowing the
            # px slot (2 banks, 8 m-tiles) and two po slots (1 bank, 4
            # m-tiles each) before their first use -- score/exp work on
            # q/k starts ~15us earlier and pass A (which never touches v)
            # keeps ScalarE busy while v streams in.
            pv_a = psX.tile([P, 8, DC], F32, tag="px", name="pv_a")
            pv_b = psO.tile([P, 4, DC], F32, tag="po", name="pv_b")
            pv_c = psO.tile([P, 4, DC], F32, tag="po", name="pv_c")

            def pv_slot(mt):
                if mt < 8:
                    return pv_a, mt
                if mt < 12:
                    return pv_b, mt - 8
                return pv_c, mt - 12

            for ko2 in range(KO // 2):
                st = stream.tile([P, 2, N], BF16, tag="stv")
                nc.sync.dma_start(
                    st[:],
                    vTin.ap()[ko2 * 2 * P : (ko2 + 1) * 2 * P, :].rearrange(
                        "(t p) n -> p t n", p=P
                    ),
                )
                for t in range(2):
                    ko = ko2 * 2 + t
                    for mt in range(NT):
                        pvt, j = pv_slot(mt)
                        # one accumulation group per bank: start only on the
                        # bank's first matmul (start zeroes the whole bank)
                        nc.tensor.matmul(
                            pvt[:, j, :],
                            st[:, t, mt * P : (mt + 1) * P],
                            wv_sb[:, ko, :],
                            start=(ko == 0 and j % 4 == 0),
                            stop=False,
                        )
            for mt in range(NT):
                pvt, j = pv_slot(mt)
                nc.tensor.matmul(
                    pvt[:, j, :],
                    ones_sb[:, :P],
                    bv_sb[:],
                    start=False,
                    stop=(j % 4 == 3),
                )
            nc.vector.tensor_copy(_r(v_sb[:, 0:8, :]), pv_a[:])
            nc.vector.tensor_copy(_r(v_sb[:, 8:12, :]), pv_b[:])
            nc.vector.tensor_copy(_r(v_sb[:, 12:16, :]), pv_c[:])

            for blk in range(4):
                px = psX.tile([DK, HC, 512], F32, tag="px", name="px")
                for ii in range(4):
                    pass_b_seg(blk, px, 4 * ii, 4 * ii + 4)
                    pass_a(4 * blk + ii)
                nb = blk * 512
                for h in range(HC):
                    nc.vector.tensor_copy(
                        _r(xT_h[h][:, nb : nb + 512]), px[:, h, :]
                    )
                for i in range(4 * blk, 4 * blk + 4):
                    out_proj(i, tail=(blk == 3))


def make_in_maps(query, key, value, Wq, bq, Wk, bk, Wv, bv, Wo):
    """Shard the full inputs into one input map per core."""
    f = lambda a: np.ascontiguousarray(np.asarray(a, dtype=np.float32))
    import ml_dtypes
    qT = np.ascontiguousarray(np.asarray(query, dtype=np.float32).T.astype(np.float16))
    kT = np.ascontiguousarray(np.asarray(key, dtype=np.float32).T.astype(np.float16))
    vT = np.ascontiguousarray(
        np.asarray(value, dtype=np.float32).T.astype(ml_dtypes.bfloat16)
    )
    Wq, bq, Wk, bk, Wv, bv, Wo = map(f, (Wq, bq, Wk, bk, Wv, bv, Wo))
    in_maps = []
    for c in range(NCORES):
        sl = slice(c * DC, (c + 1) * DC)
        shuf = lambda w: np.ascontiguousarray(
            w.reshape(KO, P, DC).transpose(1, 0, 2)
        )
        in_maps.append(
            {
                "qTin": qT,
                "kTin": kT,
                "vTin": vT,
                "wq": shuf((Wq[:, sl] / TEMP).astype(np.float16)),
                "wk": shuf(Wk[:, sl].astype(np.float16)),
                "wv": shuf(Wv[:, sl].astype(ml_dtypes.bfloat16)),
                "wo": f(Wo[sl, :]),
                "bq": f(bq[sl] / TEMP).reshape(DC, 1),
                "bk": f(bk[sl]).reshape(DC, 1),
                "bv": f(bv[sl]).reshape(1, DC),
            }
        )
    return in_maps


_NC_CACHE = []
LAST_RESULT = None


def kernel(query, key, value, Wq, bq, Wk, bk, Wv, bv, Wo, bo):
    global LAST_RESULT
    if not _NC_CACHE:
        _NC_CACHE.append(build_nc())
    nc = _NC_CACHE[0]
    in_maps = make_in_maps(query, key, value, Wq, bq, Wk, bk, Wv, bv, Wo)
    res = run_bass_kernel_spmd(nc, in_maps, core_ids=list(range(NCORES)))
    LAST_RESULT = res
    attn = np.concatenate([np.asarray(r["attn"]) for r in res.results], axis=0)
    out = np.sum([np.asarray(r["pout"]) for r in res.results], axis=0)
    out = (out + np.asarray(bo, dtype=np.float32)).astype(np.float32)
    return out, attn



# revision 42
# speedup vs baseline: 1.0383x; 1.0118x over previous
"""Multi-head attention (H=16, d_model=1024, N=2048) on 8 TRN2 NeuronCores.

Sharding: tensor-parallel over heads — each core owns 2 heads (a 128-wide
slice of the QKV projection output dim and of Wo's input dim). Activations
are replicated (shipped pre-transposed by the host); each core returns its
2 heads' attention weights and a partial output projection which the host
reduces (the tensor-parallel all-reduce).

Device pipeline per core:
  1. qT,kT = (Wslice^T @ inT) -> [128(dims), 2048(tokens)] fp32 PSUM accum
     (query/key and their weight slices ship as fp16 — >= fp32r's own
     mantissa — halving the replicated-input DMA);  v = [2048, 128] from a
     bf16 value path (linear contribution only, attn numerics untouched).
  2. pass A: scores[n,m] tiles (fp32r matmuls, 1/sqrt(dk) folded into Wq on
     host) -> exp on ScalarE with fused accum_out row-sums -> per-partition
     reciprocal scale on VectorE -> contiguous attn row DMA to HBM.
  3. pass B: scores^T[m,n] tiles -> exp -> matmul with v accumulating
     x^T[dims, n] unnormalized, 512-wide n-chunks, interleaved with pass A
     so ScalarE (the bottleneck engine: 2x 8.4M-elem exp) never idles.
  4. out_partial = x^T.T @ Wo_slice per head; the pass-A row-sum
     reciprocals are applied as per-partition scalars during PSUM eviction
     and the two heads are summed (softmax normalization commutes with the
     value/output matmuls, so it is folded into these cheap evictions
     instead of touching the 2048x2048 score matrices a second time).

Engine budget per core (cost model): DMA 155us (the 33.5MB attention
output write dominates), ScalarE 151us (near-gapless window), TensorE
123us, VectorE 100us; total ~215us = 35us q/k input prefix + exp-bound
window (the v path streams in behind the window start, its projection
borrowing the px/po PSUM slots before their first use) + 7us drain tail.
"""

import os
import sys

import numpy as np

for _p in ("/opt/trn_rl_repo", "/opt/pypackages"):
    if os.path.isdir(_p) and _p not in sys.path:
        sys.path.append(_p)

import concourse.bass as bass
import concourse.mybir as mybir
import concourse.tile as tile
from concourse import bacc
from concourse.bass_utils import run_bass_kernel_spmd

F32 = mybir.dt.float32
F32R = mybir.dt.float32r
BF16 = mybir.dt.bfloat16
F16 = mybir.dt.float16
EXP = mybir.ActivationFunctionType.Exp

N = 2048          # tokens
D = 1024          # d_model
H = 16            # heads
DK = 64           # head dim
NCORES = 8
HC = H // NCORES  # heads per core = 2
DC = HC * DK      # projection slice per core = 128
P = 128
KO = D // P       # 8 contraction tiles over d_model
NT = N // P       # 16 token tiles
TEMP = float(np.sqrt(DK))


def _r(ap):
    return ap.bitcast(F32R)


def build_nc():
    nc = bacc.Bacc(
        "TRN2", target_bir_lowering=False, debug=False, num_devices=NCORES
    )
    qTin = nc.dram_tensor("qTin", [D, N], F16, kind="ExternalInput")
    kTin = nc.dram_tensor("kTin", [D, N], F16, kind="ExternalInput")
    vTin = nc.dram_tensor("vTin", [D, N], BF16, kind="ExternalInput")
    wq = nc.dram_tensor("wq", [P, KO, DC], F16, kind="ExternalInput")
    wk = nc.dram_tensor("wk", [P, KO, DC], F16, kind="ExternalInput")
    wv = nc.dram_tensor("wv", [P, KO, DC], BF16, kind="ExternalInput")
    wo = nc.dram_tensor("wo", [DC, D], F32, kind="ExternalInput")
    bq = nc.dram_tensor("bq", [DC, 1], F32, kind="ExternalInput")
    bk = nc.dram_tensor("bk", [DC, 1], F32, kind="ExternalInput")
    bv = nc.dram_tensor("bv", [1, DC], F32, kind="ExternalInput")
    attn = nc.dram_tensor("attn", [HC, N, N], F32, kind="ExternalOutput")
    pout = nc.dram_tensor("pout", [N, D], F32, kind="ExternalOutput")

    with tile.TileContext(nc) as tc:
        _body(nc, tc, qTin, kTin, vTin, wq, wk, wv, wo, bq, bk, bv, attn, pout)
    nc.compile()
    return nc


def _body(nc, tc, qTin, kTin, vTin, wq, wk, wv, wo, bq, bk, bv, attn, pout):
    with (
        tc.tile_pool(name="wpool", bufs=1) as wpool,
        tc.tile_pool(name="big", bufs=1) as big,
        tc.tile_pool(name="stream", bufs=3) as stream,
        tc.tile_pool(name="expp", bufs=6) as expp,
        tc.tile_pool(name="etp", bufs=10) as etp,
        tc.tile_pool(name="outp", bufs=4) as outp,
        tc.tile_pool(name="small", bufs=1) as small,
    ):
        # ---- weights / constants -------------------------------------
        wq_sb = wpool.tile([P, KO, DC], F16)
        nc.sync.dma_start(wq_sb[:], wq.ap())
        wk_sb = wpool.tile([P, KO, DC], F16)
        nc.sync.dma_start(wk_sb[:], wk.ap())
        wv_sb = wpool.tile([P, KO, DC], BF16)
        nc.sync.dma_start(wv_sb[:], wv.ap())
        # per-head Wo slices at partition base 0 (DMA places partitions freely)
        wo_h = []
        for h in range(HC):
            woh = wpool.tile([DK, D], F32, name=f"wo{h}", tag=f"wo{h}")
            nc.sync.dma_start(_r(woh[:]), _r(wo.ap()[h * DK : (h + 1) * DK, :]))
            wo_h.append(woh)
        bq_sb = wpool.tile([P, 1], F32)
        nc.sync.dma_start(bq_sb[:], bq.ap())
        bk_sb = wpool.tile([P, 1], F32)
        nc.sync.dma_start(bk_sb[:], bk.ap())
        bv_sb = wpool.tile([1, DC], F32)
        nc.sync.dma_start(bv_sb[:], bv.ap())
        ones_sb = wpool.tile([1, P], F32)
        nc.vector.memset(ones_sb[:], 1.0)

        reps = int(os.environ.get("KERNEL_BENCH_REPS", "1"))
        qT_sb = big.tile([P, N], F32)   # [dims, tokens] both heads stacked
        kT_sb = big.tile([P, N], F32)
        v_sb = big.tile([P, NT, DC], F32)  # [m_in_tile, m_tile, dims]
        # per-head unnormalized attn@v, transposed, at partition base 0
        xT_h = [
            big.tile([DK, N], F32, name=f"xT{h}", tag=f"xT{h}") for h in range(HC)
        ]
        rs_sb = big.tile([P, 2 * NT], F32)  # exp row sums, col = h*NT + nt
        rr_sb = big.tile([P, 2 * NT], F32)  # reciprocals of rs_sb

        # ---- phase 1: q/k projections (v is handled in phase 2) ----
        for _rep in range(reps):
         with tc.tile_pool(name="ps1", bufs=1, space="PSUM") as ps1:
            # qT / kT: [dc, n] accumulated over 8 d_model tiles
            for name, win, wsb, bsb, dst in (
                ("q", qTin, wq_sb, bq_sb, qT_sb),
                ("k", kTin, wk_sb, bk_sb, kT_sb),
            ):
                ps_qk = ps1.tile([P, N], F32, tag="ps_qk")
                for ko2 in range(KO // 2):
                    st = stream.tile([P, 2, N], F16, tag="st")
                    nc.sync.dma_start(
                        st[:],
                        win.ap()[ko2 * 2 * P : (ko2 + 1) * 2 * P, :].rearrange(
                            "(t p) n -> p t n", p=P
                        ),
                    )
                    for t in range(2):
                        ko = ko2 * 2 + t
                        for ch in range(4):
                            nc.tensor.matmul(
                                ps_qk[:, ch * 512 : (ch + 1) * 512],
                                wsb[:, ko, :],
                                st[:, t, ch * 512 : (ch + 1) * 512],
                                start=(ko == 0),
                                stop=(ko == KO - 1),
                            )
                # chunked eviction so pass A can start per m-chunk sooner
                for ch in range(4):
                    nc.vector.tensor_scalar_add(
                        _r(dst[:, ch * 512 : (ch + 1) * 512]),
                        ps_qk[:, ch * 512 : (ch + 1) * 512],
                        bsb[:],
                    )

         # ---- phase 2+3: attention + output projection, interleaved ----
         # psA: pass-A scores [n,m] half-tiles;  psT/psX: pass-B transposed
         # scores and x^T accumulators (512-wide n chunks);  psO: out-proj.
         with (
            tc.tile_pool(name="psA", bufs=1, space="PSUM") as psA,
            tc.tile_pool(name="psT", bufs=1, space="PSUM") as psT,
            tc.tile_pool(name="psX", bufs=1, space="PSUM") as psX,
            tc.tile_pool(name="psO", bufs=2, space="PSUM") as psO,
        ):

            def pass_a(i):
                # normal-orientation attn rows nt=i for both heads
                for h in range(HC):
                    b0 = h * DK
                    qh = qT_sb[b0 : b0 + DK, :]
                    kh = kT_sb[b0 : b0 + DK, :]
                    idx = h * NT + i
                    ex = expp.tile([P, N], F32, tag="ex", name="ex")
                    rs2 = small.tile([P, 2], F32, tag="rs2", bufs=4, name="rs2")
                    for half in range(2):
                        pa = psA.tile([P, 1024], F32, tag="pa", name="pa")
                        for sub in range(2):
                            mc = half * 2 + sub
                            nc.tensor.matmul(
                                pa[:, sub * 512 : (sub + 1) * 512],
                                _r(qh[:, i * P : (i + 1) * P]),
                                _r(kh[:, mc * 512 : (mc + 1) * 512]),
                                start=True,
                                stop=True,
                            )
                        nc.scalar.activation(
                            ex[:, half * 1024 : (half + 1) * 1024],
                            pa[:],
                            EXP,
                            accum_out=rs2[:, half : half + 1],
                        )
                    nc.vector.tensor_add(
                        rs_sb[:, idx : idx + 1], rs2[:, 0:1], rs2[:, 1:2]
                    )
                    nc.vector.reciprocal(
                        rr_sb[:, idx : idx + 1], rs_sb[:, idx : idx + 1]
                    )
                    nc.vector.tensor_scalar_mul(
                        ex[:], ex[:], rr_sb[:, idx : idx + 1]
                    )
                    nc.sync.dma_start(attn.ap()[h, i * P : (i + 1) * P, :], ex[:])

            def pass_b_seg(j, px, i0, i1):
                # transposed scores for n-chunk j (512 wide), m-tiles
                # [i0,i1).  Both heads share one PSUM tile so the exp is a
                # single 1024-wide ScalarE op per m-tile.
                nb = j * 512
                for i in range(i0, i1):
                    pt = psT.tile([P, HC, 512], F32, tag="pt", name="pt")
                    for h in range(HC):
                        b0 = h * DK
                        nc.tensor.matmul(
                            pt[:, h, :],
                            _r(kT_sb[b0 : b0 + DK, i * P : (i + 1) * P]),
                            _r(qT_sb[b0 : b0 + DK, nb : nb + 512]),
                            start=True,
                            stop=True,
                        )
                    et = etp.tile([P, HC, 512], F32, tag="et", name="et")
                    nc.scalar.activation(_r(et[:]), pt[:], EXP)
                    for h in range(HC):
                        b0 = h * DK
                        nc.tensor.matmul(
                            px[:, h, :],
                            _r(v_sb[:, i, b0 : b0 + DK]),
                            _r(et[:, h, :]),
                            start=(i == 0),
                            stop=(i == NT - 1),
                        )

            def out_proj(i, tail=False):
                # out rows nt=i: matmul per head, normalize by pass-A row
                # sums during eviction, combine, store. In the kernel tail
                # (last block) spread the evictions over the idle ACT and
                # GpSimd engines instead of serializing on DVE.
                for ch in range(2):
                    po = [None, None]
                    for h in range(HC):
                        po[h] = psO.tile([P, 512], F32, tag="po", name=f"po{h}")
                        nc.tensor.matmul(
                            po[h][:],
                            _r(xT_h[h][:, i * P : (i + 1) * P]),
                            _r(wo_h[h][:, ch * 512 : (ch + 1) * 512]),
                            start=True,
                            stop=True,
                        )
                    ob = outp.tile([P, 512], F32, tag="ob", name="ob")
                    nc.vector.tensor_scalar_mul(ob[:], po[0][:], rr_sb[:, i : i + 1])
                    ob2 = outp.tile([P, 512], F32, tag="ob2", name="ob2")
                    if tail:
                        nc.scalar.mul(ob2[:], po[1][:], rr_sb[:, NT + i : NT + i + 1])
                        nc.gpsimd.tensor_add(ob[:], ob[:], ob2[:])
                    else:
                        nc.vector.tensor_scalar_mul(
                            ob2[:], po[1][:], rr_sb[:, NT + i : NT + i + 1]
                        )
                        nc.vector.tensor_add(ob[:], ob[:], ob2[:])
                    nc.sync.dma_start(
                        pout.ap()[i * P : (i + 1) * P, ch * 512 : (ch + 1) * 512],
                        ob[:],
                    )

            # interleave: alternate pass-A tiles with pass-B segments so
            # ACT always has ready work; out-proj as soon as its n-chunk of
            # x^T completes
            # v projection inside the attention scope, borrowing the
            # px slot (2 banks, 8 m-tiles) and two po slots (1 bank, 4
            # m-tiles each) before their first use -- score/exp work on
            # q/k starts ~15us earlier and pass A (which never touches v)
            # keeps ScalarE busy while v streams in.
            pv_a = psX.tile([P, 8, DC], F32, tag="px", name="pv_a")
            pv_b = psO.tile([P, 4, DC], F32, tag="po", name="pv_b")
            pv_c = psO.tile([P, 4, DC], F32, tag="po", name="pv_c")

            def pv_slot(mt):
                if mt < 8:
                    return pv_a, mt
                if mt < 12:
                    return pv_b, mt - 8
                return pv_c, mt - 12

            for ko2 in range(KO // 2):
                st = stream.tile([P, 2, N], BF16, tag="stv")
                nc.sync.dma_start(
                    st[:],
                    vTin.ap()[ko2 * 2 * P : (ko2 + 1) * 2 * P, :].rearrange(
                        "(t p) n -> p t n", p=P
                    ),
                )
                for t in range(2):
                    ko = ko2 * 2 + t
                    for mt in range(NT):
                        pvt, j = pv_slot(mt)
                        # one accumulation group per bank: start only on the
                        # bank's first matmul (start zeroes the whole bank)
                        nc.tensor.matmul(
                            pvt[:, j, :],
                            st[:, t, mt * P : (mt + 1) * P],
                            wv_sb[:, ko, :],
                            start=(ko == 0 and j % 4 == 0),
                            stop=False,
                        )
            for mt in range(NT):
                pvt, j = pv_slot(mt)
                nc.tensor.matmul(
                    pvt[:, j, :],
                    ones_sb[:, :P],
                    bv_sb[:],
                    start=False,
                    stop=(j % 4 == 3),
                )
            nc.vector.tensor_copy(_r(v_sb[:, 0:8, :]), pv_a[:])
            nc.vector.tensor_copy(_r(v_sb[:, 8:12, :]), pv_b[:])
            nc.vector.tensor_copy(_r(v_sb[:, 12:16, :]), pv_c[:])

            for blk in range(4):
                px = psX.tile([DK, HC, 512], F32, tag="px", name="px")
                for ii in range(4):
                    pass_b_seg(blk, px, 4 * ii, 4 * ii + 4)
                    pass_a(4 * blk + ii)
                nb = blk * 512
                for h in range(HC):
                    nc.vector.tensor_copy(
                        _r(xT_h[h][:, nb : nb + 512]), px[:, h, :]
                    )
                for i in range(4 * blk, 4 * blk + 4):
                    out_proj(i, tail=(blk == 3))


def make_in_maps(query, key, value, Wq, bq, Wk, bk, Wv, bv, Wo):
    """Shard the full inputs into one input map per core."""
    f = lambda a: np.ascontiguousarray(np.asarray(a, dtype=np.float32))
    import ml_dtypes
    qT = np.ascontiguousarray(np.asarray(query, dtype=np.float32).T.astype(np.float16))
    kT = np.ascontiguousarray(np.asarray(key, dtype=np.float32).T.astype(np.float16))
    vT = np.ascontiguousarray(
        np.asarray(value, dtype=np.float32).T.astype(ml_dtypes.bfloat16)
    )
    Wq, bq, Wk, bk, Wv, bv, Wo = map(f, (Wq, bq, Wk, bk, Wv, bv, Wo))
    in_maps = []
    for c in range(NCORES):
        sl = slice(c * DC, (c + 1) * DC)
        shuf = lambda w: np.ascontiguousarray(
            w.reshape(KO, P, DC).transpose(1, 0, 2)
        )
        in_maps.append(
            {
                "qTin": qT,
                "kTin": kT,
                "vTin": vT,
                "wq": shuf((Wq[:, sl] / TEMP).astype(np.float16)),
                "wk": shuf(Wk[:, sl].astype(np.float16)),
                "wv": shuf(Wv[:, sl].astype(ml_dtypes.bfloat16)),
                "wo": f(Wo[sl, :]),
                "bq": f(bq[sl] / TEMP).reshape(DC, 1),
                "bk": f(bk[sl]).reshape(DC, 1),
                "bv": f(bv[sl]).reshape(1, DC),
            }
        )
    return in_maps


_NC_CACHE = []
LAST_RESULT = None


def kernel(query, key, value, Wq, bq, Wk, bk, Wv, bv, Wo, bo):
    global LAST_RESULT
    if not _NC_CACHE:
        _NC_CACHE.append(build_nc())
    nc = _NC_CACHE[0]
    in_maps = make_in_maps(query, key, value, Wq, bq, Wk, bk, Wv, bv, Wo)
    res = run_bass_kernel_spmd(nc, in_maps, core_ids=list(range(NCORES)))
    LAST_RESULT = res
    attn = np.concatenate([np.asarray(r["attn"]) for r in res.results], axis=0)
    out = np.sum([np.asarray(r["pout"]) for r in res.results], axis=0)
    out = (out + np.asarray(bo, dtype=np.float32)).astype(np.float32)
    return out, attn

